# revision 1
# baseline (speedup 1.0000x reference)
"""Causal self-attention (B=4, S=2048, D=768, H=12) on 8 TRN2 NeuronCores.

Sharding: batch (4) x head-group (2) = 8 cores.  Each core computes, for its
batch b and 6 heads:
  - x^T via PE transposes (contraction over D needs D on partitions)
  - QK^T projection directly in transposed layout (head-dim on partitions),
    Q pre-scaled by 1/sqrt(dh) via host-side weight scaling
  - V projection in natural layout, with a ones column appended per head
    (so the AV matmul also produces softmax denominators for free)
  - flash-style causal attention with scores kept transposed
    (S^T = K Q^T): softmax needs no max-subtraction (scores are O(1) here),
    exp on ACT, causal mask as a 0/1 multiply on diagonal blocks only
  - AV^T accumulated in PSUM over key chunks -> O^T [dh, q] per head,
    normalized by PE-broadcast reciprocal of the fused sums row
  - partial output projection (its 384 rows of W_out)
Host: sums the two partial outputs per batch and adds the constant
b_v @ W_out + b_out (V-bias commutes through softmax-normalized attention).

All matmuls run in float32r (full-rate fp32 on the PE, ~1.2e-4 rounding).
"""

import numpy as np

import concourse.bass as bass
import concourse.tile as tile
import concourse.mybir as mybir
from concourse import bacc
from concourse._compat import with_exitstack  # noqa: F401  (parity with repo kernels)

F32 = mybir.dt.float32
F32R = mybir.dt.float32r

B, S, D = 4, 2048, 768
H, DH = 12, 64
G = 2                 # head groups (tensor-parallel dimension)
HPG = H // G          # heads per group = 6
NPAIR = HPG // 2      # head pairs per group = 3
N_CORES = 8
ST = 128              # S-tile for projections / output rows
QT = 512              # q-tile for attention
KC = 128              # key chunk
N_ST = S // ST        # 16
N_QT = S // QT        # 4
DC = D // 128         # 6 contraction chunks over D


def declare_io(nc):
    """DRAM tensors; names must match in_maps keys."""
    io = {}
    io["x"] = nc.dram_tensor("x", [S, D], F32R, kind="ExternalInput")
    io["wqk"] = nc.dram_tensor("wqk", [D, 768], F32R, kind="ExternalInput")
    io["bqk2"] = nc.dram_tensor("bqk2", [128, 6], F32, kind="ExternalInput")
    io["wv"] = nc.dram_tensor("wv", [D, 384], F32R, kind="ExternalInput")
    io["wo"] = nc.dram_tensor("wo", [384, 768], F32R, kind="ExternalInput")
    io["masks"] = nc.dram_tensor("masks", [2, KC, QT], F32R, kind="ExternalInput")
    io["ident"] = nc.dram_tensor("ident", [128, 128], F32R, kind="ExternalInput")
    io["sel"] = nc.dram_tensor("sel", [128, 128], F32R, kind="ExternalInput")
    io["ones2"] = nc.dram_tensor("ones2", [128, HPG], F32R, kind="ExternalInput")
    io["out"] = nc.dram_tensor("out", [S, D], F32, kind="ExternalOutput")
    return io


def build_body(nc, tc, pools, io, phases=(1, 2, 3, 4, 5)):
    """Emit one full forward pass (per-core program)."""
    (consts, w768, wsmall, slab, qkT_p, vsb_p, xload, psA, psB, scp, pT_p,
     rcp_p, atmp_p, outsb_p) = pools

    # ---- constants / weights into SBUF ----
    ident_t = consts.tile([128, 128], F32R, tag="ident")
    nc.sync.dma_start(out=ident_t, in_=io["ident"][:])
    sel_t = consts.tile([128, 128], F32R, tag="sel")
    nc.sync.dma_start(out=sel_t, in_=io["sel"][:])
    bqk2_t = consts.tile([128, 6], F32, tag="bqk2")
    nc.sync.dma_start(out=bqk2_t, in_=io["bqk2"][:])
    masks_t = []
    for r in range(2):
        m = consts.tile([KC, QT], F32R, tag=f"mask{r}")
        nc.sync.dma_start(out=m, in_=io["masks"][r])
        masks_t.append(m)

    wqk_t = []
    for c in range(DC):
        w = w768.tile([128, 768], F32R, tag="w768")
        nc.sync.dma_start(out=w, in_=io["wqk"][c * 128:(c + 1) * 128, :])
        wqk_t.append(w)
    wv_t = []
    for c in range(DC):
        w = wsmall.tile([128, 384], F32R, tag="wv")
        nc.sync.dma_start(out=w, in_=io["wv"][c * 128:(c + 1) * 128, :])
        wv_t.append(w)

    # ---- x^T (6 tiles [128, S]) via PE transposes, 4 S-tiles per copy ----
    xT = [slab.tile([128, S], F32R, tag="slab", name=f"xT{c}") for c in range(DC)]
    for s4 in range(N_ST // 4):
        xts = []
        for si in range(4):
            s = s4 * 4 + si
            xt = xload.tile([128, D], F32R, tag="xload")
            nc.sync.dma_start(out=xt, in_=io["x"][s * ST:(s + 1) * ST, :])
            xts.append(xt)
        for c in range(DC):
            tp = psA.tile([128, 512], F32R, tag="ps1")
            for si in range(4):
                nc.tensor.transpose(tp[:, si * 128:(si + 1) * 128],
                                    xts[si][:, c * 128:(c + 1) * 128], ident_t[:])
            nc.vector.tensor_copy(xT[c][:, s4 * 512:(s4 + 1) * 512], tp)

    if 2 not in phases:
        return
    # ---- QK^T projection: qkT[j] j even = Q-pair j//2, j odd = K-pair ----
    qkT = [qkT_p.tile([128, S], F32R, tag="qkT", name=f"qkT{j}") for j in range(6)]
    for j in range(6):
        for t in range(N_QT):
            pp = psA.tile([128, QT], F32, tag="ps1")
            for c in range(DC):
                nc.tensor.matmul(pp, wqk_t[c][:, j * 128:(j + 1) * 128],
                                 xT[c][:, t * QT:(t + 1) * QT],
                                 start=(c == 0), stop=(c == DC - 1))
            nc.vector.tensor_scalar_add(qkT[j][:, t * QT:(t + 1) * QT], pp,
                                        bqk2_t[:, j:j + 1])

    if 3 not in phases:
        return
    # ---- V projection into [V_h | ones] blocks of 65 cols ----
    vsb = []
    for s in range(N_ST):
        vp = psA.tile([128, 384], F32, tag="ps1")
        for c in range(DC):
            nc.tensor.matmul(vp, xT[c][:, s * ST:(s + 1) * ST], wv_t[c][:],
                             start=(c == 0), stop=(c == DC - 1))
        vv = vsb_p.tile([128, HPG, 65], F32R, tag="vsb")
        nc.vector.tensor_copy(vv[:, :, 0:64],
                              vp[:].rearrange("p (h d) -> p h d", h=HPG))
        nc.sync.dma_start(out=vv[:, :, 64:65],
                          in_=io["ones2"][:].rearrange("p (h o) -> p h o", o=1))
        vsb.append(vv)

    if 4 not in phases:
        return
    # ---- attention ----
    apair = [slab.tile([128, S], F32R, tag="slab", name=f"apair{p}") for p in range(NPAIR)]
    for p in range(NPAIR):
        qp = qkT[2 * p]
        kp = qkT[2 * p + 1]
        for t in range(N_QT):
            n_kc = 4 * t + 4
            av_e = psB.tile([65, QT], F32, tag="ps2")
            av_o = psB.tile([65, QT], F32, tag="ps2")
            avs = (av_e, av_o)

            def chunk_geom(kc):
                # causal slicing: diagonal chunk r only touches q-window
                # [off, 512); r==3 widened to 256 so fp32r stays full-rate.
                r = kc - 4 * t
                if r < 0:
                    return 0, QT, None
                if r < 3:
                    return 128 * r, QT - 128 * r, masks_t[0][:, 0:128]
                return 256, 256, masks_t[1][:, 0:256]

            for kc in range(n_kc):
                off, w, msk = chunk_geom(kc)
                # both heads' scores in one 2-bank PSUM tile so exp and the
                # causal-mask multiply run once per chunk pair (ACT per-op
                # overhead is ~300ns; halving the op count matters)
                sc2 = scp.tile([KC, 2, QT], F32, tag="sc2")
                pt2 = pT_p.tile([KC, 2, QT], F32R, tag="pT")
                for j in (0, 1):
                    nc.tensor.matmul(
                        sc2[:, j, 0:w],
                        kp[j * 64:(j + 1) * 64, kc * KC:(kc + 1) * KC],
                        qp[j * 64:(j + 1) * 64, t * QT + off:(t + 1) * QT],
                        start=True, stop=True, tile_position=(j * 64, 0))
                nc.scalar.activation(pt2[:, :, 0:w], sc2[:, :, 0:w],
                                     mybir.ActivationFunctionType.Exp)
                if msk is not None:
                    mw = msk.shape[1]
                    msk2 = bass.AP(tensor=msk.tensor, offset=msk.offset,
                                   ap=[list(msk.ap[0]), [0, 2], list(msk.ap[1])])
                    nc.vector.tensor_mul(pt2[:, :, 0:mw], pt2[:, :, 0:mw], msk2)
                for j in (0, 1):
                    nc.tensor.matmul(
                        avs[j][:, off:off + w], vsb[kc][:, 2 * p + j, :],
                        pt2[:, j, 0:w],
                        start=(kc == 0), stop=(kc == n_kc - 1))
            rc_e = rcp_p.tile([65, QT], F32R, tag="rcp")
            nc.vector.reciprocal(rc_e[64:65, :], av_e[64:65, :])
            rc_o = rcp_p.tile([65, QT], F32R, tag="rcp")
            nc.vector.reciprocal(rc_o[64:65, :], av_o[64:65, :])
            bc_e = psA.tile([64, QT], F32, tag="ps1")
            nc.tensor.matmul(bc_e, sel_t[64:65, 0:64], rc_e[64:65, :],
                             start=True, stop=True)
            bc_e_sb = rcp_p.tile([64, QT], F32, tag="bcsb")
            nc.vector.tensor_copy(bc_e_sb, bc_e)
            bc_o = psA.tile([64, QT], F32, tag="ps1")
            nc.tensor.matmul(bc_o, sel_t[64:65, 0:64], rc_o[64:65, :],
                             start=True, stop=True)
            bc_o_sb = rcp_p.tile([64, QT], F32, tag="bcsb")
            nc.vector.tensor_copy(bc_o_sb, bc_o)
            nc.vector.tensor_mul(apair[p][0:64, t * QT:(t + 1) * QT],
                                 av_e[0:64, :], bc_e_sb[:])
            at = atmp_p.tile([64, QT], F32R, tag="atmp")
            nc.vector.tensor_mul(at, av_o[0:64, :], bc_o_sb[:])
            nc.sync.dma_start(out=apair[p][64:128, t * QT:(t + 1) * QT], in_=at)

    if 5 not in phases:
        return
    # ---- output projection (partial: this group's 384 rows of W_out) ----
    wo_t = []
    for p in range(NPAIR):
        w = w768.tile([128, 768], F32R, tag="w768")
        nc.sync.dma_start(out=w, in_=io["wo"][p * 128:(p + 1) * 128, :])
        wo_t.append(w)
    for s in range(N_ST):
        o1 = psA.tile([128, 512], F32, tag="ps1")
        o2 = psA.tile([128, 256], F32, tag="ps1")
        for p in range(NPAIR):
            nc.tensor.matmul(o1, apair[p][:, s * ST:(s + 1) * ST],
                             wo_t[p][:, 0:512],
                             start=(p == 0), stop=(p == NPAIR - 1))
        for p in range(NPAIR):
            nc.tensor.matmul(o2, apair[p][:, s * ST:(s + 1) * ST],
                             wo_t[p][:, 512:768],
                             start=(p == 0), stop=(p == NPAIR - 1))
        osb = outsb_p.tile([128, D], F32, tag="outsb")
        nc.vector.tensor_copy(osb[:, 0:512], o1)
        nc.vector.tensor_copy(osb[:, 512:768], o2)
        nc.sync.dma_start(out=io["out"][s * ST:(s + 1) * ST, :], in_=osb)


def make_pools(tc, ctx):
    consts = ctx.enter_context(tc.tile_pool(name="consts", bufs=1))
    w768 = ctx.enter_context(tc.tile_pool(name="w768", bufs=6))
    wsmall = ctx.enter_context(tc.tile_pool(name="wsmall", bufs=6))
    slab = ctx.enter_context(tc.tile_pool(name="slab", bufs=6))
    qkT_p = ctx.enter_context(tc.tile_pool(name="qkT", bufs=6))
    vsb_p = ctx.enter_context(tc.tile_pool(name="vsb", bufs=16))
    xload = ctx.enter_context(tc.tile_pool(name="xload", bufs=5))
    psA = ctx.enter_context(tc.tile_pool(name="psA", bufs=2, space="PSUM"))
    psB = ctx.enter_context(tc.tile_pool(name="psB", bufs=2, space="PSUM"))
    scp = ctx.enter_context(tc.tile_pool(name="scp", bufs=2, space="PSUM"))
    pT_p = ctx.enter_context(tc.tile_pool(name="pT", bufs=3))
    rcp_p = ctx.enter_context(tc.tile_pool(name="rcp", bufs=2))
    atmp_p = ctx.enter_context(tc.tile_pool(name="atmp", bufs=2))
    outsb_p = ctx.enter_context(tc.tile_pool(name="outsb", bufs=2))
    return (consts, w768, wsmall, slab, qkT_p, vsb_p, xload, psA, psB, scp,
            pT_p, rcp_p, atmp_p, outsb_p)


def build_nc(n_iters=None, phases=(1, 2, 3, 4, 5)):
    """Build the per-core program. n_iters wraps the body in a HW loop
    (timing harness only; the graded path uses n_iters=None)."""
    from contextlib import ExitStack

    nc = bacc.Bacc(trn_type="TRN2", debug=False)
    nc._allow_low_precision_reason = "float32r matmuls keep fp32 width"
    io = declare_io(nc)
    with tile.TileContext(nc) as tc:
        with ExitStack() as ctx:
            pools = make_pools(tc, ctx)
            if n_iters is None:
                build_body(nc, tc, pools, io, phases)
            else:
                with tc.For_i(0, n_iters, 1):
                    build_body(nc, tc, pools, io, phases)
    nc.compile()
    return nc, io


def host_inputs(x, W_qkv, b_qkv, W_out, b_out):
    """Per-core in_maps + the host-side unshard constant."""
    x = np.asarray(x, dtype=np.float32)
    W_qkv = np.asarray(W_qkv, dtype=np.float32)
    b_qkv = np.asarray(b_qkv, dtype=np.float32)
    W_out = np.asarray(W_out, dtype=np.float32)
    b_out = np.asarray(b_out, dtype=np.float32)

    Wq, Wk, Wv = W_qkv[:, 0:D], W_qkv[:, D:2 * D], W_qkv[:, 2 * D:3 * D]
    bq, bk, bv = b_qkv[0:D], b_qkv[D:2 * D], b_qkv[2 * D:3 * D]
    scale = 1.0 / np.sqrt(DH)

    # shared constants
    masks = np.zeros((2, KC, QT), np.float32)
    for r in range(2):
        kk = np.arange(KC)[:, None]
        qq = np.arange(QT)[None, :]
        masks[r] = (qq >= kk + KC * r).astype(np.float32)
    ident = np.eye(128, dtype=np.float32)
    sel = np.zeros((128, 128), np.float32)
    sel[64, 0:64] = 1.0
    ones2 = np.ones((128, HPG), np.float32)

    per_group = []
    for g in range(G):
        cols = []
        bcols = []
        for p in range(NPAIR):
            h0 = g * HPG + 2 * p
            h1 = h0 + 1
            cols.append(Wq[:, h0 * DH:(h0 + 2) * DH] * scale)   # q-pair
            cols.append(Wk[:, h0 * DH:(h0 + 2) * DH])           # k-pair
            bcols.append(bq[h0 * DH:(h0 + 2) * DH] * scale)
            bcols.append(bk[h0 * DH:(h0 + 2) * DH])
        wqk_g = np.concatenate(cols, axis=1)                    # [768, 768]
        bqk_g = np.stack(bcols, axis=1)                         # [128, 6]
        wv_g = Wv[:, g * HPG * DH:(g + 1) * HPG * DH]           # [768, 384]
        wo_g = W_out[g * HPG * DH:(g + 1) * HPG * DH, :]        # [384, 768]
        per_group.append((wqk_g, bqk_g, wv_g, wo_g))

    in_maps = []
    for core in range(N_CORES):
        b, g = core // G, core % G
        wqk_g, bqk_g, wv_g, wo_g = per_group[g]
        in_maps.append(dict(
            x=np.ascontiguousarray(x[b]),
            wqk=np.ascontiguousarray(wqk_g),
            bqk2=np.ascontiguousarray(bqk_g),
            wv=np.ascontiguousarray(wv_g),
            wo=np.ascontiguousarray(wo_g),
            masks=masks, ident=ident, sel=sel,
            ones2=ones2,
        ))
    cvec = (bv @ W_out + b_out).astype(np.float32)              # [768]
    return in_maps, cvec


_CACHE = {}


def kernel(x, W_qkv, b_qkv, W_out, b_out):
    from concourse.bass_utils import run_bass_kernel_spmd

    if "nc" not in _CACHE:
        _CACHE["nc"], _ = build_nc()
    nc = _CACHE["nc"]
    in_maps, cvec = host_inputs(x, W_qkv, b_qkv, W_out, b_out)
    res = run_bass_kernel_spmd(nc, in_maps, list(range(N_CORES)))
    out = np.empty((B, S, D), np.float32)
    for b in range(B):
        out[b] = res.results[2 * b]["out"] + res.results[2 * b + 1]["out"] + cvec
    return out



# revision 18
# speedup vs baseline: 1.4844x; 1.4844x over previous
"""Causal self-attention (B=4, S=2048, D=768, H=12) on 8 TRN2 NeuronCores.

Sharding: batch (4) x head-group (2) = 8 cores.  Each core computes, for its
batch b and 6 heads, the full attention + its partial output projection.

v2 design (vs v1 baseline):
  - x is transposed and cast to bf16 on the host ("xt" input [768, 2048]);
    no PE transposes, no x^T copies on DVE, 6 big DMAs instead of 64.
  - all matmul operands are bf16 (1 cycle/row on the PE at any width, same
    rate as float32r at >=256 but without the small-tile penalty);
    accumulation stays fp32 in PSUM.
  - scores kept transposed (S^T = K Q^T, keys on partitions); exp on ACT
    (bf16 output), causal mask as a 0/1 bf16 multiply on the first 128
    query columns of diagonal chunks only.
  - AV matmuls fused with softmax denominators via a ones column per head;
    AV emission delayed one chunk behind the score stream so the PE never
    waits on ACT in steady state.
  - normalization: DVE reciprocal of the fused sums row, partition
    broadcast of the reciprocal row on the (otherwise idle) Pool/GPSIMD
    engine, even-head normalize on DVE straight out of PSUM, odd-head rows
    moved to partitions 64:128 with a DVE stream_shuffle and normalized on
    Pool.  Out-projection PSUM drains split between ACT (copy shares the
    exp table, no reload) and DVE.
  - out-projection tiles are emitted inside the attention normalization
    windows so the PE has fill work while an AV bank drains (PSUM is fully
    booked: 4 banks scores double-buffer, 2 banks AV, 2 banks projections).
Host: sums the two partial outputs per batch and adds the constant
b_v @ W_out + b_out (V-bias commutes through softmax-normalized attention).
"""

import numpy as np
import ml_dtypes

import concourse.bass as bass
import concourse.tile as tile
import concourse.mybir as mybir
from concourse import bacc

F32 = mybir.dt.float32
F32R = mybir.dt.float32r
BF16 = mybir.dt.bfloat16
EXP = mybir.ActivationFunctionType.Exp
COPY = mybir.ActivationFunctionType.Copy

B, S, D = 4, 2048, 768
H, DH = 12, 64
G = 2                 # head groups (tensor-parallel dimension)
HPG = H // G          # heads per group = 6
NPAIR = HPG // 2      # head pairs per group = 3
N_CORES = 8
ST = 128              # S-tile for projections / output rows
QT = 512              # q-tile for attention
KC = 128              # key chunk
N_ST = S // ST        # 16
N_QT = S // QT        # 4
DC = D // 128         # 6 contraction chunks over D

IDENT32 = list(range(32))


def declare_io(nc):
    io = {}
    io["xt"] = nc.dram_tensor("xt", [D, S], BF16, kind="ExternalInput")
    io["wqk"] = nc.dram_tensor("wqk", [D, 768], BF16, kind="ExternalInput")
    io["bqk2"] = nc.dram_tensor("bqk2", [128, 6], F32, kind="ExternalInput")
    io["wv"] = nc.dram_tensor("wv", [D, 384], BF16, kind="ExternalInput")
    io["wo"] = nc.dram_tensor("wo", [384, 768], BF16, kind="ExternalInput")
    io["mask"] = nc.dram_tensor("mask", [KC, KC], BF16, kind="ExternalInput")
    io["ones1"] = nc.dram_tensor("ones1", [1, 64], F32R, kind="ExternalInput")
    io["ones2"] = nc.dram_tensor("ones2", [128, HPG], BF16, kind="ExternalInput")
    io["out"] = nc.dram_tensor("out", [S, D], F32, kind="ExternalOutput")
    return io


def make_pools(tc, ctx):
    consts = ctx.enter_context(tc.tile_pool(name="consts", bufs=1))
    wqk_p = ctx.enter_context(tc.tile_pool(name="wqk", bufs=DC))
    wv_p = ctx.enter_context(tc.tile_pool(name="wv", bufs=DC))
    wo_p = ctx.enter_context(tc.tile_pool(name="wo", bufs=NPAIR))
    xt_p = ctx.enter_context(tc.tile_pool(name="xt", bufs=DC))
    qkT_p = ctx.enter_context(tc.tile_pool(name="qkT", bufs=6))
    vsb_p = ctx.enter_context(tc.tile_pool(name="vsb", bufs=N_ST))
    apair_p = ctx.enter_context(tc.tile_pool(name="apair", bufs=NPAIR))
    pt_p = ctx.enter_context(tc.tile_pool(name="pt", bufs=3))
    rc_p = ctx.enter_context(tc.tile_pool(name="rc", bufs=4))
    bc_p = ctx.enter_context(tc.tile_pool(name="bc", bufs=2))
    ato_p = ctx.enter_context(tc.tile_pool(name="ato", bufs=2))
    ate_p = ctx.enter_context(tc.tile_pool(name="ate", bufs=2))
    at2_p = ctx.enter_context(tc.tile_pool(name="at2", bufs=2))
    osb_p = ctx.enter_context(tc.tile_pool(name="osb", bufs=2))
    psA = ctx.enter_context(tc.tile_pool(name="psA", bufs=2, space="PSUM"))
    scp = ctx.enter_context(tc.tile_pool(name="scp", bufs=2, space="PSUM"))
    avp = ctx.enter_context(tc.tile_pool(name="avp", bufs=1, space="PSUM"))
    return (consts, wqk_p, wv_p, wo_p, xt_p, qkT_p, vsb_p, apair_p, pt_p,
            rc_p, bc_p, ato_p, ate_p, at2_p, osb_p, psA, scp, avp)


def build_body(nc, tc, pools, io):
    (consts, wqk_p, wv_p, wo_p, xt_p, qkT_p, vsb_p, apair_p, pt_p,
     rc_p, bc_p, ato_p, ate_p, at2_p, osb_p, psA, scp, avp) = pools

    # ---- constants / weights / x^T into SBUF ----
    mask_t = consts.tile([KC, KC], BF16, tag="mask")
    nc.sync.dma_start(out=mask_t, in_=io["mask"][:])
    bqk2_t = consts.tile([128, HPG], F32, tag="bqk2")
    nc.sync.dma_start(out=bqk2_t, in_=io["bqk2"][:])
    ones_row = consts.tile([1, 64], F32R, tag="ones1")
    nc.sync.dma_start(out=ones_row, in_=io["ones1"][:])
    ones2_t = consts.tile([128, HPG], BF16, tag="ones2")
    nc.sync.dma_start(out=ones2_t, in_=io["ones2"][:])

    wqk_t = []
    for c in range(DC):
        w = wqk_p.tile([128, 768], BF16, tag="wqk")
        nc.sync.dma_start(out=w, in_=io["wqk"][c * 128:(c + 1) * 128, :])
        wqk_t.append(w)
    wv_t = []
    for c in range(DC):
        w = wv_p.tile([128, 384], BF16, tag="wv")
        nc.sync.dma_start(out=w, in_=io["wv"][c * 128:(c + 1) * 128, :])
        wv_t.append(w)
    wo_t = []
    for p in range(NPAIR):
        w = wo_p.tile([128, 768], BF16, tag="wo")
        nc.sync.dma_start(out=w, in_=io["wo"][p * 128:(p + 1) * 128, :])
        wo_t.append(w)
    xt_t = []
    for c in range(DC):
        x = xt_p.tile([128, S], BF16, tag="xt")
        nc.sync.dma_start(out=x, in_=io["xt"][c * 128:(c + 1) * 128, :])
        xt_t.append(x)

    # mask AP broadcast over the 2-head axis: [128, (0,2), 128]
    m = mask_t[:]
    mask2 = bass.AP(tensor=m.tensor, offset=m.offset,
                    ap=[list(m.ap[0]), [0, 2], list(m.ap[1])])

    qkT = [qkT_p.tile([128, S], BF16, tag="qkT", name=f"qkT{j}")
           for j in range(2 * NPAIR)]
    apair = [apair_p.tile([128, S], BF16, tag="apair", name=f"apair{p}")
             for p in range(NPAIR)]
    vsb = [None] * N_ST

    def emit_qk_pair(p):
        # qkT[2p] = scaled Q pair, qkT[2p+1] = K pair (dh on partitions)
        for j in (2 * p, 2 * p + 1):
            for t in range(N_QT):
                pp = psA.tile([128, QT], F32, tag="ps1")
                for c in range(DC):
                    nc.tensor.matmul(pp, wqk_t[c][:, j * 128:(j + 1) * 128],
                                     xt_t[c][:, t * QT:(t + 1) * QT],
                                     start=(c == 0), stop=(c == DC - 1))
                nc.vector.tensor_scalar_add(qkT[j][:, t * QT:(t + 1) * QT],
                                            pp, bqk2_t[:, j:j + 1])

    def emit_v(s):
        # V rows for s-tile s, [V_h | ones] blocks of 65 cols per head
        vp = psA.tile([128, QT], F32, tag="ps1")
        for c in range(DC):
            nc.tensor.matmul(vp[:, 0:384], xt_t[c][:, s * ST:(s + 1) * ST],
                             wv_t[c][:], start=(c == 0), stop=(c == DC - 1))
        vv = vsb_p.tile([128, HPG, 65], BF16, tag="vsb")
        nc.vector.tensor_copy(vv[:, :, 0:64],
                              vp[:, 0:384].rearrange("p (h d) -> p h d", h=HPG))
        nc.vector.tensor_copy(vv[:, :, 64:65],
                              ones2_t[:].rearrange("p (h o) -> p h o", o=1))
        vsb[s] = vv

    def emit_o(s):
        # partial output projection for s-tile s (this group's 384 W_out rows)
        o1 = psA.tile([128, QT], F32, tag="ps1")
        o2 = psA.tile([128, QT], F32, tag="ps1")
        for p3 in range(NPAIR):
            nc.tensor.matmul(o1[:, 0:512], apair[p3][:, s * ST:(s + 1) * ST],
                             wo_t[p3][:, 0:512],
                             start=(p3 == 0), stop=(p3 == NPAIR - 1))
        for p3 in range(NPAIR):
            nc.tensor.matmul(o2[:, 0:256], apair[p3][:, s * ST:(s + 1) * ST],
                             wo_t[p3][:, 512:768],
                             start=(p3 == 0), stop=(p3 == NPAIR - 1))
        osb = osb_p.tile([128, D], F32, tag="osb")
        nc.scalar.activation(osb[:, 0:512], o1[:, 0:512], COPY)
        nc.vector.tensor_copy(osb[:, 512:768], o2[:, 0:256])
        nc.sync.dma_start(out=io["out"][s * ST:(s + 1) * ST, :], in_=osb)

    def emit_att(p, t, pe_filler=None):
        qp = qkT[2 * p]
        kp = qkT[2 * p + 1]
        n_kc = 4 * t + 4
        # av: [0:65, 0, :] even head (V rows 0..63 + denominator row 64),
        #     [0:65, 1, :] odd head.  One 2-bank PSUM tile.
        avt = avp.tile([128, 2, QT], F32, tag="av")
        pts = [None] * n_kc

        def geom(kc):
            r = kc - 4 * t
            if r < 0:
                return 0, QT, False
            return 128 * r, QT - 128 * r, True

        def emit_sc(kc):
            off, w, diag = geom(kc)
            sc2 = scp.tile([KC, 2, QT], F32, tag="sc")
            pt2 = pt_p.tile([KC, 2, QT], BF16, tag="pt")
            for j in (0, 1):
                nc.tensor.matmul(
                    sc2[:, j, 0:w],
                    kp[j * 64:(j + 1) * 64, kc * KC:(kc + 1) * KC],
                    qp[j * 64:(j + 1) * 64, t * QT + off:(t + 1) * QT],
                    start=True, stop=True, tile_position=(j * 64, 0))
            nc.scalar.activation(pt2[:, :, 0:w], sc2[:, :, 0:w], EXP)
            if diag:
                # diagonal block is always the first 128 query columns
                nc.vector.tensor_mul(pt2[:, :, 0:KC], pt2[:, :, 0:KC], mask2)
            pts[kc] = (pt2, off, w)

        def emit_av(kc):
            pt2, off, w = pts[kc]
            for j in (0, 1):
                nc.tensor.matmul(avt[0:65, j, off:off + w],
                                 vsb[kc][:, 2 * p + j, :], pt2[:, j, 0:w],
                                 start=(kc == 0), stop=(kc == n_kc - 1))
            pts[kc] = None

        # AV one chunk behind the score stream: PE runs sc(kc+1) while ACT
        # computes exp(kc), then drains av(kc) with inputs already ready.
        emit_sc(0)
        for kc in range(1, n_kc):
            emit_sc(kc)
            emit_av(kc - 1)
        emit_av(n_kc - 1)

        # normalization.  Free the AV PSUM bank with the shortest possible
        # DVE chain: copy both heads (+ denominator rows) to SBUF, then
        # reciprocal / broadcast / multiply off the critical path.  The
        # reciprocal rows are PE-broadcast (ones-row matmul, baseline-proven)
        # and the normalize multiplies read the broadcast straight from PSUM
        # (tensor_tensor allows one PSUM operand), so no bc copy is needed.
        ate = ate_p.tile([65, QT], F32, tag="ate")
        nc.vector.tensor_copy(ate, avt[0:65, 0, :])
        ato = ato_p.tile([65, QT], F32, tag="ato")
        nc.vector.tensor_copy(ato, avt[0:65, 1, :])
        rc_e = rc_p.tile([1, QT], F32R, tag="rc")
        nc.vector.reciprocal(rc_e, ate[64:65, :])
        rc_o = rc_p.tile([1, QT], F32R, tag="rc")
        nc.vector.reciprocal(rc_o, ato[64:65, :])
        if pe_filler is not None:
            pe_filler()  # out-proj matmuls fill the PE while the bank drains
        bc_e = psA.tile([128, QT], F32, tag="ps1")
        nc.tensor.matmul(bc_e[0:64, :], ones_row[:], rc_e[:],
                         start=True, stop=True, tile_position=(0, 0))
        bc_o = psA.tile([128, QT], F32, tag="ps1")
        nc.tensor.matmul(bc_o[0:64, :], ones_row[:], rc_o[:],
                         start=True, stop=True, tile_position=(0, 0))
        cols = slice(t * QT, (t + 1) * QT)
        nc.vector.tensor_mul(apair[p][0:64, cols], ate[0:64, :], bc_e[0:64, :])
        at2 = at2_p.tile([64, QT], BF16, tag="at2")
        nc.vector.tensor_mul(at2, ato[0:64, :], bc_o[0:64, :])
        nc.sync.dma_start(out=apair[p][64:128, cols], in_=at2)

    # ---- schedule ----
    emit_qk_pair(0)
    for s in range(0, 4):
        emit_v(s)
    emit_att(0, 0)
    emit_qk_pair(1)
    for s in range(4, 8):
        emit_v(s)
    emit_att(1, 0)
    emit_qk_pair(2)
    for s in range(8, 12):
        emit_v(s)
    emit_att(2, 0)
    for s in range(12, 16):
        emit_v(s)

    from collections import deque
    ready_o = deque([0, 1, 2, 3])
    for t in range(1, N_QT):
        for p in range(NPAIR):
            # deeper fill at the last pair of each round (2 o-tiles)
            n_fill = 2 if p == NPAIR - 1 else 1
            ss = [ready_o.popleft() for _ in range(min(n_fill, len(ready_o)))]
            filler = None
            if ss:
                def filler(ss=ss):
                    for s in ss:
                        emit_o(s)
            emit_att(p, t, filler)
        ready_o.extend(range(4 * t, 4 * t + 4))
    for s in ready_o:
        emit_o(s)


def build_nc(n_iters=None):
    """Build the per-core program. n_iters wraps the body in a HW loop
    (timing harness only; the graded path uses n_iters=None)."""
    from contextlib import ExitStack

    nc = bacc.Bacc(trn_type="TRN2", debug=False)
    nc._allow_low_precision_reason = (
        "bf16 matmul inputs with fp32 PSUM accumulation; tolerance 2e-2"
    )
    io = declare_io(nc)
    with tile.TileContext(nc) as tc:
        with ExitStack() as ctx:
            pools = make_pools(tc, ctx)
            if n_iters is None:
                build_body(nc, tc, pools, io)
            else:
                with tc.For_i(0, n_iters, 1):
                    build_body(nc, tc, pools, io)
    nc.compile()
    return nc, io


def host_inputs(x, W_qkv, b_qkv, W_out, b_out):
    """Per-core in_maps + the host-side unshard constant."""
    bf16 = ml_dtypes.bfloat16
    x = np.asarray(x, dtype=np.float32)
    W_qkv = np.asarray(W_qkv, dtype=np.float32)
    b_qkv = np.asarray(b_qkv, dtype=np.float32)
    W_out = np.asarray(W_out, dtype=np.float32)
    b_out = np.asarray(b_out, dtype=np.float32)

    Wq, Wk, Wv = W_qkv[:, 0:D], W_qkv[:, D:2 * D], W_qkv[:, 2 * D:3 * D]
    bq, bk, bv = b_qkv[0:D], b_qkv[D:2 * D], b_qkv[2 * D:3 * D]
    scale = 1.0 / np.sqrt(DH)

    kk = np.arange(KC)[:, None]
    cc = np.arange(KC)[None, :]
    mask = (cc >= kk).astype(bf16)

    per_group = []
    for g in range(G):
        cols = []
        bcols = []
        for p in range(NPAIR):
            h0 = g * HPG + 2 * p
            cols.append(Wq[:, h0 * DH:(h0 + 2) * DH] * scale)   # q-pair
            cols.append(Wk[:, h0 * DH:(h0 + 2) * DH])           # k-pair
            bcols.append(bq[h0 * DH:(h0 + 2) * DH] * scale)
            bcols.append(bk[h0 * DH:(h0 + 2) * DH])
        wqk_g = np.concatenate(cols, axis=1).astype(bf16)       # [768, 768]
        bqk_g = np.stack(bcols, axis=1).astype(np.float32)      # [128, 6]
        wv_g = Wv[:, g * HPG * DH:(g + 1) * HPG * DH].astype(bf16)
        wo_g = W_out[g * HPG * DH:(g + 1) * HPG * DH, :].astype(bf16)
        per_group.append((wqk_g, bqk_g, wv_g, wo_g))

    xt_b = [np.ascontiguousarray(x[b].T).astype(bf16) for b in range(B)]

    in_maps = []
    for core in range(N_CORES):
        b, g = core // G, core % G
        wqk_g, bqk_g, wv_g, wo_g = per_group[g]
        in_maps.append(dict(
            xt=xt_b[b],
            wqk=np.ascontiguousarray(wqk_g),
            bqk2=np.ascontiguousarray(bqk_g),
            wv=np.ascontiguousarray(wv_g),
            wo=np.ascontiguousarray(wo_g),
            mask=mask,
            ones1=np.ones((1, 64), np.float32),
            ones2=np.ones((128, HPG), bf16),
        ))
    cvec = (bv @ W_out + b_out).astype(np.float32)              # [768]
    return in_maps, cvec


_CACHE = {}


def kernel(x, W_qkv, b_qkv, W_out, b_out):
    from concourse.bass_utils import run_bass_kernel_spmd

    if "nc" not in _CACHE:
        _CACHE["nc"], _ = build_nc()
    nc = _CACHE["nc"]
    in_maps, cvec = host_inputs(x, W_qkv, b_qkv, W_out, b_out)
    res = run_bass_kernel_spmd(nc, in_maps, list(range(N_CORES)))
    out = np.empty((B, S, D), np.float32)
    for b in range(B):
        out[b] = res.results[2 * b]["out"] + res.results[2 * b + 1]["out"] + cvec
    return out
